# revision 21
# baseline (speedup 1.0000x reference)
"""Trainium2 Bass kernel for nms_detection (scatter-mean -> sigmoid -> YOLOX decode).

Strategy
--------
Data-parallel over the batch axis: core c owns batches [4c, 4c+4).  The
scatter-mean (segment mean of ~7M node vectors into dense per-scale grids) is
reformulated as a dense segment-sum done by the PE array:

  * Host bins nodes by destination cell and splits each cell's nodes into
    RN=4-node chunks.  Cells are sorted per-core by chunk count J
    (descending) and packed into groups of 2304 (72 columns x 32 slots x 4
    rows); chunk level j of a group only spans the prefix of cells that
    still have a j-th chunk, so each level's matmul uses a variable width
    and almost no zero padding is shipped (fill ~0.95).  Levels accumulate
    into the group's PSUM bank via start/stop flags (start zeroes the 2KB
    zero region; partial-width accumulation is legal).  The fixed 0/1
    indicator weight W[k, m] = (k//4 == m) is padded to 128 columns.
  * Values ship as fp8 e3m4 (4-bit mantissa; the 2e-2 output tolerance
    absorbs the ~2^-5 quantization error), streamed as big supertiles
    alternating across both HWDGE rings; matmul slices that straddle a
    supertile boundary split into two accumulating matmuls.  A dense burst
    of tiny warm-up matmuls runs while the first supertile streams in,
    pushing the PE activity monitor to full clock early.
  * Sums are copied (vector engine) from PSUM[0:32] into a [128, nb*504]
    fp32 accumulator (partition stripe = group%4 * 32).  Per 4-group block,
    the decode epilogue computes xy = sum*(rec*s) + grid*s,
    wh = exp(sum*rec)*s, and sigmoid = 1/(1 + exp(sum*rec)) with obj/cls
    negated host-side and the reciprocal done by the DVE's fast approx op —
    so the ACT engine only ever needs the Exp table (no activation-table
    reloads).  Results are written as fp16 and DMAd out per block; the host
    reassembles [32, 6300, 7] in fp32.
"""

import numpy as np
import ml_dtypes

import concourse.bacc as bacc
import concourse.mybir as mybir
import concourse.tile as tile
from concourse.bass_utils import run_bass_kernel_spmd

# Problem geometry (fixed by the nn.Module spec).
B = 32
NCORES = 8
GRIDS = [(60, 80), (30, 40), (15, 20)]
STRIDES = [3.0, 6.0, 12.0]
CHD = 7            # channels per cell: reg(4) | obj(1) | cls(2)
COUT = 7

# Device layout knobs.
RN = 4             # nodes per cell chunk (contraction rows per cell slot)
G = 128 // RN      # cell slots per contraction block = 32
GPB = 128 // G     # groups per output partition block = 4
CB = 72            # cell columns per group
TILE_F = CB * CHD  # 504 = PSUM tile free size
CPG = CB * G       # cells per group = 2304
NK = 6             # fp16 constants per cell: gx*s, gy*s, rec*s, rec, s, 1.0
SUP = 8192         # supertile width (bytes per partition)
RAMP = [2048, 4096]  # initial supertile widths (quick pipeline start)
NWARM = 24         # PE clock-ramp warm-up matmuls

_f32 = mybir.dt.float32
_f16 = mybir.dt.float16
_f8 = mybir.dt.float8e3
_np_f8 = ml_dtypes.float8_e3m4


def _ceil_div(a, b):
    return (a + b - 1) // b


def _prep(inputs):
    """Host preprocessing: bin nodes by cell, build the packed fp8 stream."""
    bpc = B // NCORES
    HWs = [h * w for h, w in GRIDS]
    cell_base = np.concatenate([[0], np.cumsum([B * hw for hw in HWs])])
    a_off = np.concatenate([[0], np.cumsum(HWs)])
    ncells = int(cell_base[-1])
    ncpc = ncells // NCORES

    cnt_all = np.zeros(ncells, np.int64)
    coc_all = np.empty(ncells, np.int64)
    gxs_all = np.empty(ncells, np.float32)
    gys_all = np.empty(ncells, np.float32)
    st_all = np.empty(ncells, np.float32)
    bcell_all = np.empty(ncells, np.int64)
    anch_all = np.empty(ncells, np.int64)

    node_cell, node_rank, node_val = [], [], []
    for s in range(3):
        H, W = GRIDS[s]
        HW = H * W
        stride = np.float32(STRIDES[s])
        pos = np.asarray(inputs[f"pos{s + 1}"], np.float32)
        batch = np.asarray(inputs[f"batch{s + 1}"]).astype(np.int64)
        col = np.clip((pos[:, 0] / stride).astype(np.int32), 0, W - 1)
        row = np.clip((pos[:, 1] / stride).astype(np.int32), 0, H - 1)
        lid = batch * HW + row * W + col
        n = lid.shape[0]
        cnt = np.bincount(lid, minlength=B * HW)
        order = np.argsort(lid, kind="stable")
        starts = np.zeros(B * HW + 1, np.int64)
        np.cumsum(cnt, out=starts[1:])
        rank = np.empty(n, np.int64)
        rank[order] = np.arange(n, dtype=np.int64) - starts[lid[order]]
        node_cell.append(cell_base[s] + lid)
        node_rank.append(rank)
        vals = np.concatenate(
            [
                np.asarray(inputs[f"reg{s + 1}"], np.float32),
                np.asarray(inputs[f"obj{s + 1}"], np.float32),
                np.asarray(inputs[f"cls{s + 1}"], np.float32),
            ],
            axis=1,
        )
        vals[:, 4:7] *= -1.0  # sigmoid(m) computed as 1/(1+exp(-m))
        node_val.append(vals)

        sl = slice(int(cell_base[s]), int(cell_base[s + 1]))
        cnt_all[sl] = cnt
        a = np.arange(B * HW, dtype=np.int64) % HW
        b = np.arange(B * HW, dtype=np.int64) // HW
        coc_all[sl] = b // bpc
        gxs_all[sl] = (a % W).astype(np.float32) * stride
        gys_all[sl] = (a // W).astype(np.float32) * stride
        st_all[sl] = stride
        bcell_all[sl] = b
        anch_all[sl] = a_off[s] + a

    J_all = np.maximum(1, _ceil_div(cnt_all, RN))  # chunk levels per cell
    Jmax = int(J_all.max())
    ng = _ceil_div(ncpc, CPG)
    nb = _ceil_div(ng, GPB)

    # per-core sort by J descending (stable), groups of CPG cells
    key = coc_all * (Jmax + 1) + (Jmax - J_all)
    order = np.argsort(key, kind="stable")
    rank_core = np.arange(ncells, dtype=np.int64) - coc_all[order] * ncpc
    g_c = np.empty(ncells, np.int64)
    cb_c = np.empty(ncells, np.int64)
    m_c = np.empty(ncells, np.int64)
    g_c[order] = rank_core // CPG
    u = rank_core % CPG
    cb_c[order] = u // G
    m_c[order] = u % G

    # level width profiles: n_j[c, g, j] = #cells in (c, g) with J >= j
    cnt3 = np.zeros((NCORES, ng, Jmax + 2), np.int64)
    np.add.at(cnt3, (coc_all, g_c, J_all), 1)
    rc = cnt3[:, :, ::-1].cumsum(axis=2)[:, :, ::-1]
    rcmax = rc.max(axis=0)                     # max over cores [ng, Jmax+2]
    Jg = (rcmax[:, 1:] > 0).sum(axis=1)        # levels per group
    wmax = _ceil_div(rcmax, G)                 # width in cells per J-threshold

    bases, widths = [], []
    c0 = 0
    for g in range(ng):
        bg, wg = [], []
        for j0 in range(int(Jg[g])):
            w = CB if j0 == 0 else int(wmax[g, j0 + 1])
            bg.append(c0)
            wg.append(w)
            c0 += w * CHD
        bases.append(bg)
        widths.append(wg)
    TOTC = _ceil_div(c0, 16) * 16

    base_arr = np.zeros((ng, Jmax + 1), np.int64)
    for g in range(ng):
        for j0 in range(int(Jg[g])):
            base_arr[g, j0] = bases[g][j0]

    # node placement into the packed per-core stream
    xall = np.zeros((NCORES, 128 * TOTC), _np_f8)
    ch7 = np.arange(CHD, dtype=np.int64)
    for s in range(3):
        gc = node_cell[s]
        rk = node_rank[s]
        off = (
            (m_c[gc] * RN + rk % RN) * TOTC
            + base_arr[g_c[gc], rk // RN]
            + cb_c[gc] * CHD
        )
        xall[coc_all[gc][:, None], off[:, None] + ch7] = node_val[s].astype(_np_f8)
    xall = xall.reshape(NCORES, 128, TOTC)

    # per-cell decode constants in fp16
    rec = np.float32(1.0) / np.maximum(cnt_all, 1).astype(np.float32)
    prow = (g_c % GPB) * G + m_c
    ccol = (g_c // GPB) * (CB * NK) + cb_c * NK
    cdat = np.zeros((NCORES, 128, nb * CB * NK), np.float16)
    cdat[coc_all, prow, ccol + 0] = gxs_all
    cdat[coc_all, prow, ccol + 1] = gys_all
    cdat[coc_all, prow, ccol + 2] = rec * st_all
    cdat[coc_all, prow, ccol + 3] = rec
    cdat[coc_all, prow, ccol + 4] = st_all
    cdat[coc_all, prow, ccol + 5] = 1.0

    wmat = np.zeros((128, 128), _np_f8)
    wmat[np.arange(128), np.arange(128) // RN] = 1.0  # cols G..127 stay zero

    # supertile schedule and matmul piece program
    sts = []
    c = 0
    i = 0
    while c < TOTC:
        w = RAMP[i] if i < len(RAMP) else SUP
        sts.append((c, min(TOTC, c + w)))
        c += w
        i += 1
    st_starts = np.array([a for a, _ in sts])

    prog = []
    for g in range(ng):
        gp = []
        for j0 in range(int(Jg[g])):
            cb0 = bases[g][j0]
            cb1 = cb0 + widths[g][j0] * CHD
            cc = cb0
            while cc < cb1:
                si = int(np.searchsorted(st_starts, cc, side="right") - 1)
                s0, s1 = sts[si]
                ee = min(cb1, s1)
                gp.append(
                    (
                        si,
                        cc - s0,
                        cc - cb0,
                        ee - cc,
                        j0 == 0 and cc == cb0,
                        j0 == int(Jg[g]) - 1 and ee == cb1,
                    )
                )
                cc = ee
        prog.append(gp)

    meta = dict(
        ng=ng, nb=nb, TOTC=TOTC, sts=sts, prog=prog,
        coc=coc_all, prow=prow,
        fcol=(g_c // GPB) * TILE_F + cb_c * CHD,
        bcell=bcell_all, anch=anch_all,
    )
    in_maps = [
        {"xd": xall[c], "wd": wmat, "cd": cdat[c]} for c in range(NCORES)
    ]
    return meta, in_maps


def _build(meta):
    """Build the SPMD Bass program (identical for all cores)."""
    ng = meta["ng"]
    nb = meta["nb"]
    TOTC = meta["TOTC"]
    sts = meta["sts"]
    prog = meta["prog"]

    nc = bacc.Bacc(trn_type="TRN2", target_bir_lowering=False, debug=False)
    xd = nc.dram_tensor("xd", [128, TOTC], _f8, kind="ExternalInput")
    wd = nc.dram_tensor("wd", [128, 128], _f8, kind="ExternalInput")
    cd = nc.dram_tensor("cd", [128, nb * CB * NK], _f16, kind="ExternalInput")
    outd = nc.dram_tensor("out", [128, nb * TILE_F], _f16, kind="ExternalOutput")

    act = mybir.ActivationFunctionType
    alu = mybir.AluOpType

    with tile.TileContext(nc) as tc:
        with (
            tc.tile_pool(name="const", bufs=1) as cpool,
            tc.tile_pool(name="xin", bufs=14) as xpool,
            tc.tile_pool(name="acc", bufs=1) as apool,
            tc.tile_pool(name="ps", bufs=8, space="PSUM") as ppool,
        ):
            wsb = cpool.tile([128, 128], _f8)
            nc.sync.dma_start(out=wsb[:], in_=wd[:])
            csb = cpool.tile([128, nb * CB * NK], _f16)
            nc.gpsimd.dma_start(out=csb[:], in_=cd[:])
            osb = apool.tile([128, nb * TILE_F], _f32)
            obf = apool.tile([128, nb * TILE_F], _f16)
            sigt = apool.tile([128, CB * 3], _f32)  # partition-0 recip scratch

            # warm the Exp table and the PE activity monitor while the first
            # supertiles stream in
            warm = cpool.tile([128, 8], _f32)
            nc.vector.memset(warm[:], 0.0)
            nc.scalar.activation(warm[:], warm[:], act.Exp)
            for _ in range(NWARM):
                wps = ppool.tile([128, TILE_F], _f32, tag="ps")
                nc.tensor.matmul(
                    out=wps[:, :128], lhsT=wsb[:], rhs=wsb[:],
                    start=True, stop=True,
                )

            # each supertile ships as two column-half DMAs, one per HWDGE
            # ring, so both rings advance the same column position in lockstep
            supers = []
            for i, (c0, c1) in enumerate(sts):
                xt = xpool.tile([128, SUP], _f8, tag="xin")
                w = c1 - c0
                wh = _ceil_div(w // 2, 16) * 16
                nc.sync.dma_start(out=xt[:, :wh], in_=xd[:, c0 : c0 + wh])
                nc.scalar.dma_start(out=xt[:, wh:w], in_=xd[:, c0 + wh : c1])
                supers.append(xt)

            def finish_block(b, p0, P):
                fs = slice(b * TILE_F, (b + 1) * TILE_F)
                v = osb[p0:P, fs].rearrange("p (q c) -> p q c", c=CHD)
                o = obf[p0:P, fs].rearrange("p (q c) -> p q c", c=CHD)
                cv = csb[p0:P, b * (CB * NK) : (b + 1) * (CB * NK)].rearrange(
                    "p (q k) -> p q k", k=NK
                )
                P = P - p0
                # xy = sum*(rec*s) + grid*s
                nc.vector.tensor_tensor(
                    out=o[:, :, 0:2], in0=v[:, :, 0:2],
                    in1=cv[:, :, 2:3].to_broadcast((P, CB, 2)), op=alu.mult,
                )
                nc.vector.tensor_tensor(
                    out=o[:, :, 0:2], in0=o[:, :, 0:2],
                    in1=cv[:, :, 0:2], op=alu.add,
                )
                # wh = exp(sum*rec) * s   (means are < 10, clip never binds)
                nc.vector.tensor_tensor(
                    out=v[:, :, 2:4], in0=v[:, :, 2:4],
                    in1=cv[:, :, 3:4].to_broadcast((P, CB, 2)), op=alu.mult,
                )
                nc.scalar.activation(v[:, :, 2:4], v[:, :, 2:4], act.Exp)
                nc.vector.tensor_tensor(
                    out=o[:, :, 2:4], in0=v[:, :, 2:4],
                    in1=cv[:, :, 4:5].to_broadcast((P, CB, 2)), op=alu.mult,
                )
                # sigmoid(m) = 1 / (1 + exp(-m)); obj/cls pre-negated
                nc.vector.tensor_tensor(
                    out=v[:, :, 4:7], in0=v[:, :, 4:7],
                    in1=cv[:, :, 3:4].to_broadcast((P, CB, 3)), op=alu.mult,
                )
                nc.scalar.activation(v[:, :, 4:7], v[:, :, 4:7], act.Exp)
                # the custom-DVE reciprocal requires partition-base-0 APs, so
                # the +1 lands in a base-0 scratch and the copy shifts back
                sg = sigt[0:P, :].rearrange("p (q c) -> p q c", c=3)
                nc.vector.tensor_tensor(
                    out=sg, in0=v[:, :, 4:7],
                    in1=cv[:, :, 5:6].to_broadcast((P, CB, 3)), op=alu.add,
                )
                nc.vector.reciprocal_approx_fast(out=sg, in_=sg)
                nc.vector.tensor_copy(out=o[:, :, 4:7], in_=sg)
                nc.sync.dma_start(
                    out=outd[p0 : p0 + P, fs], in_=obf[p0 : p0 + P, fs]
                )

            for g in range(ng):
                ps = ppool.tile([128, TILE_F], _f32, tag="ps")
                for si, soff, ooff, wc, fstart, fstop in prog[g]:
                    nc.tensor.matmul(
                        out=ps[:, ooff : ooff + wc],
                        lhsT=wsb[:],
                        rhs=supers[si][:, soff : soff + wc],
                        start=fstart,
                        stop=fstop,
                    )
                pb = (g % GPB) * G
                b = g // GPB
                nc.vector.tensor_copy(
                    out=osb[pb : pb + G, b * TILE_F : (b + 1) * TILE_F],
                    in_=ps[0:G, :],
                )
                if b == nb - 1:
                    # last block: per-stripe epilogue/output so the tail only
                    # waits on the final group's 32-partition chain
                    finish_block(b, pb, pb + G)
                elif g % GPB == GPB - 1:
                    finish_block(b, 0, GPB * G)
    nc.compile()
    return nc


def _assemble(meta, outs):
    """Host-side gather of the per-core device outputs into [B, A, 7]."""
    total_a = sum(h * w for h, w in GRIDS)
    oc = np.stack(outs).astype(np.float32)  # [NCORES, 128, nb*TILE_F]
    ch = np.arange(COUT, dtype=np.int64)
    vals = oc[
        meta["coc"][:, None], meta["prow"][:, None], meta["fcol"][:, None] + ch
    ]
    final = np.empty((B, total_a, COUT), np.float32)
    final[meta["bcell"], meta["anch"]] = vals
    return final


def _run(inputs, trace=False, trace_cores=None):
    meta, in_maps = _prep(inputs)
    nc = _build(meta)
    kwargs = {}
    if trace:
        kwargs = dict(trace=True)
        if trace_cores is not None:
            kwargs["trace_cores"] = trace_cores
    res = run_bass_kernel_spmd(
        nc, in_maps, core_ids=list(range(NCORES)), **kwargs
    )
    out = _assemble(meta, [r["out"] for r in res.results])
    return out, res


def kernel(**inputs) -> np.ndarray:
    out, _ = _run(inputs, trace=False)
    return out


# revision 25
# speedup vs baseline: 1.1081x; 1.1081x over previous
"""Trainium2 Bass kernel for nms_detection (scatter-mean -> sigmoid -> YOLOX decode).

Strategy
--------
Data-parallel over the batch axis: core c owns batches [4c, 4c+4).  The
scatter-mean (segment mean of ~7M node vectors into dense per-scale grids) is
reformulated as a dense segment-sum done by the PE array:

  * Host bins nodes by destination cell and splits each cell's nodes into
    RN=4-node chunks.  Cells are sorted per-core by chunk count J
    (descending) and packed into groups of 2304 (72 columns x 32 slots x 4
    rows); chunk level j of a group only spans the prefix of cells that
    still have a j-th chunk, so each level's matmul uses a variable width
    and almost no zero padding is shipped (fill ~0.95).  Levels accumulate
    into the group's PSUM bank via start/stop flags (start zeroes the 2KB
    zero region; partial-width accumulation is legal).  The fixed 0/1
    indicator weight W[k, m] = (k//4 == m) is padded to 128 columns.
  * Values ship as fp8 e3m4 (4-bit mantissa; the 2e-2 output tolerance
    absorbs the ~2^-5 quantization error), streamed as big supertiles
    alternating across both HWDGE rings; matmul slices that straddle a
    supertile boundary split into two accumulating matmuls.  A dense burst
    of tiny warm-up matmuls runs while the first supertile streams in,
    pushing the PE activity monitor to full clock early.
  * Sums are copied (vector engine) from PSUM[0:32] into a [128, nb*504]
    fp32 accumulator (partition stripe = group%4 * 32).  Per 4-group block,
    the decode epilogue computes xy = sum*(rec*s) + grid*s,
    wh = exp(sum*rec)*s, and sigmoid = 1/(1 + exp(sum*rec)) with obj/cls
    negated host-side and the reciprocal done by the DVE's fast approx op —
    so the ACT engine only ever needs the Exp table (no activation-table
    reloads).  Results are written as fp16 and DMAd out per block; the host
    reassembles [32, 6300, 7] in fp32.
"""

import numpy as np
import ml_dtypes

import concourse.bacc as bacc
import concourse.mybir as mybir
import concourse.tile as tile
from concourse.bass_utils import run_bass_kernel_spmd

# Problem geometry (fixed by the nn.Module spec).
B = 32
NCORES = 8
GRIDS = [(60, 80), (30, 40), (15, 20)]
STRIDES = [3.0, 6.0, 12.0]
CHD = 7            # channels per cell: reg(4) | obj(1) | cls(2)
COUT = 7

# Device layout knobs.
RN = 4             # nodes per cell chunk (contraction rows per cell slot)
G = 128 // RN      # cell slots per contraction block = 32
GPB = 128 // G     # groups per output partition block = 4
CB = 72            # cell columns per group
TILE_F = CB * CHD  # 504 = PSUM tile free size
CPG = CB * G       # cells per group = 2304
NK = 6             # fp16 constants per cell: gx*s, gy*s, rec*s, rec, s, 1.0
SUP = 8192         # supertile width (bytes per partition)
RAMP = [2048, 4096]  # initial supertile widths (quick pipeline start)
NWARM = 48         # PE clock-ramp warm-up matmuls

_f32 = mybir.dt.float32
_f16 = mybir.dt.float16
_f8 = mybir.dt.float8e3
_np_f8 = ml_dtypes.float8_e3m4


def _ceil_div(a, b):
    return (a + b - 1) // b


def _prep(inputs):
    """Host preprocessing: bin nodes by cell, build the packed fp8 stream."""
    bpc = B // NCORES
    HWs = [h * w for h, w in GRIDS]
    cell_base = np.concatenate([[0], np.cumsum([B * hw for hw in HWs])])
    a_off = np.concatenate([[0], np.cumsum(HWs)])
    ncells = int(cell_base[-1])
    ncpc = ncells // NCORES

    cnt_all = np.zeros(ncells, np.int64)
    coc_all = np.empty(ncells, np.int64)
    gxs_all = np.empty(ncells, np.float32)
    gys_all = np.empty(ncells, np.float32)
    st_all = np.empty(ncells, np.float32)
    bcell_all = np.empty(ncells, np.int64)
    anch_all = np.empty(ncells, np.int64)

    node_cell, node_rank, node_val = [], [], []
    for s in range(3):
        H, W = GRIDS[s]
        HW = H * W
        stride = np.float32(STRIDES[s])
        pos = np.asarray(inputs[f"pos{s + 1}"], np.float32)
        batch = np.asarray(inputs[f"batch{s + 1}"]).astype(np.int64)
        col = np.clip((pos[:, 0] / stride).astype(np.int32), 0, W - 1)
        row = np.clip((pos[:, 1] / stride).astype(np.int32), 0, H - 1)
        lid = batch * HW + row * W + col
        n = lid.shape[0]
        cnt = np.bincount(lid, minlength=B * HW)
        order = np.argsort(lid, kind="stable")
        starts = np.zeros(B * HW + 1, np.int64)
        np.cumsum(cnt, out=starts[1:])
        rank = np.empty(n, np.int64)
        rank[order] = np.arange(n, dtype=np.int64) - starts[lid[order]]
        node_cell.append(cell_base[s] + lid)
        node_rank.append(rank)
        vals = np.concatenate(
            [
                np.asarray(inputs[f"reg{s + 1}"], np.float32),
                np.asarray(inputs[f"obj{s + 1}"], np.float32),
                np.asarray(inputs[f"cls{s + 1}"], np.float32),
            ],
            axis=1,
        )
        vals[:, 4:7] *= -1.0  # sigmoid(m) computed as 1/(1+exp(-m))
        node_val.append(vals)

        sl = slice(int(cell_base[s]), int(cell_base[s + 1]))
        cnt_all[sl] = cnt
        a = np.arange(B * HW, dtype=np.int64) % HW
        b = np.arange(B * HW, dtype=np.int64) // HW
        coc_all[sl] = b // bpc
        gxs_all[sl] = (a % W).astype(np.float32) * stride
        gys_all[sl] = (a // W).astype(np.float32) * stride
        st_all[sl] = stride
        bcell_all[sl] = b
        anch_all[sl] = a_off[s] + a

    J_all = np.maximum(1, _ceil_div(cnt_all, RN))  # chunk levels per cell
    Jmax = int(J_all.max())
    ng = _ceil_div(ncpc, CPG)
    nb = _ceil_div(ng, GPB)

    # per-core sort by J descending (stable), groups of CPG cells
    key = coc_all * (Jmax + 1) + (Jmax - J_all)
    order = np.argsort(key, kind="stable")
    rank_core = np.arange(ncells, dtype=np.int64) - coc_all[order] * ncpc
    g_c = np.empty(ncells, np.int64)
    cb_c = np.empty(ncells, np.int64)
    m_c = np.empty(ncells, np.int64)
    g_c[order] = rank_core // CPG
    u = rank_core % CPG
    cb_c[order] = u // G
    m_c[order] = u % G

    # level width profiles: n_j[c, g, j] = #cells in (c, g) with J >= j
    cnt3 = np.zeros((NCORES, ng, Jmax + 2), np.int64)
    np.add.at(cnt3, (coc_all, g_c, J_all), 1)
    rc = cnt3[:, :, ::-1].cumsum(axis=2)[:, :, ::-1]
    rcmax = rc.max(axis=0)                     # max over cores [ng, Jmax+2]
    Jg = (rcmax[:, 1:] > 0).sum(axis=1)        # levels per group
    wmax = _ceil_div(rcmax, G)                 # width in cells per J-threshold

    bases, widths = [], []
    c0 = 0
    for g in range(ng):
        bg, wg = [], []
        for j0 in range(int(Jg[g])):
            w = CB if j0 == 0 else int(wmax[g, j0 + 1])
            bg.append(c0)
            wg.append(w)
            c0 += w * CHD
        bases.append(bg)
        widths.append(wg)
    TOTC = _ceil_div(c0, 16) * 16

    base_arr = np.zeros((ng, Jmax + 1), np.int64)
    for g in range(ng):
        for j0 in range(int(Jg[g])):
            base_arr[g, j0] = bases[g][j0]

    # node placement into the packed per-core stream
    xall = np.zeros((NCORES, 128 * TOTC), _np_f8)
    ch7 = np.arange(CHD, dtype=np.int64)
    for s in range(3):
        gc = node_cell[s]
        rk = node_rank[s]
        off = (
            (m_c[gc] * RN + rk % RN) * TOTC
            + base_arr[g_c[gc], rk // RN]
            + cb_c[gc] * CHD
        )
        xall[coc_all[gc][:, None], off[:, None] + ch7] = node_val[s].astype(_np_f8)
    xall = xall.reshape(NCORES, 128, TOTC)

    # per-cell decode constants in fp16
    rec = np.float32(1.0) / np.maximum(cnt_all, 1).astype(np.float32)
    prow = (g_c % GPB) * G + m_c
    ccol = (g_c // GPB) * (CB * NK) + cb_c * NK
    cdat = np.zeros((NCORES, 128, nb * CB * NK), np.float16)
    cdat[coc_all, prow, ccol + 0] = gxs_all
    cdat[coc_all, prow, ccol + 1] = gys_all
    cdat[coc_all, prow, ccol + 2] = rec * st_all
    cdat[coc_all, prow, ccol + 3] = rec
    cdat[coc_all, prow, ccol + 4] = st_all
    cdat[coc_all, prow, ccol + 5] = 1.0

    wmat = np.zeros((128, 128), _np_f8)
    wmat[np.arange(128), np.arange(128) // RN] = 1.0  # cols G..127 stay zero

    # supertile schedule and matmul piece program
    sts = []
    c = 0
    i = 0
    while c < TOTC:
        w = RAMP[i] if i < len(RAMP) else SUP
        sts.append((c, min(TOTC, c + w)))
        c += w
        i += 1
    st_starts = np.array([a for a, _ in sts])

    prog = []
    for g in range(ng):
        gp = []
        for j0 in range(int(Jg[g])):
            cb0 = bases[g][j0]
            cb1 = cb0 + widths[g][j0] * CHD
            cc = cb0
            while cc < cb1:
                si = int(np.searchsorted(st_starts, cc, side="right") - 1)
                s0, s1 = sts[si]
                ee = min(cb1, s1)
                gp.append(
                    (
                        si,
                        cc - s0,
                        cc - cb0,
                        ee - cc,
                        j0 == 0 and cc == cb0,
                        j0 == int(Jg[g]) - 1 and ee == cb1,
                    )
                )
                cc = ee
        prog.append(gp)

    meta = dict(
        ng=ng, nb=nb, TOTC=TOTC, sts=sts, prog=prog,
        coc=coc_all, prow=prow,
        fcol=(g_c // GPB) * TILE_F + cb_c * CHD,
        bcell=bcell_all, anch=anch_all,
    )
    in_maps = [
        {"xd": xall[c], "wd": wmat, "cd": cdat[c]} for c in range(NCORES)
    ]
    return meta, in_maps


def _build(meta):
    """Build the SPMD Bass program (identical for all cores)."""
    ng = meta["ng"]
    nb = meta["nb"]
    TOTC = meta["TOTC"]
    sts = meta["sts"]
    prog = meta["prog"]

    nc = bacc.Bacc(trn_type="TRN2", target_bir_lowering=False, debug=False)
    xd = nc.dram_tensor("xd", [128, TOTC], _f8, kind="ExternalInput")
    wd = nc.dram_tensor("wd", [128, 128], _f8, kind="ExternalInput")
    cd = nc.dram_tensor("cd", [128, nb * CB * NK], _f16, kind="ExternalInput")
    outd = nc.dram_tensor("out", [128, nb * TILE_F], _f16, kind="ExternalOutput")

    act = mybir.ActivationFunctionType
    alu = mybir.AluOpType

    with tile.TileContext(nc) as tc:
        with (
            tc.tile_pool(name="const", bufs=1) as cpool,
            tc.tile_pool(name="xin", bufs=14) as xpool,
            tc.tile_pool(name="acc", bufs=1) as apool,
            tc.tile_pool(name="ps", bufs=8, space="PSUM") as ppool,
        ):
            wsb = cpool.tile([128, 128], _f8)
            nc.sync.dma_start(out=wsb[:], in_=wd[:])
            csb = cpool.tile([128, nb * CB * NK], _f16)
            nc.gpsimd.dma_start(out=csb[:], in_=cd[:])
            osb = apool.tile([128, nb * TILE_F], _f32)
            obf = apool.tile([128, nb * TILE_F], _f16)
            sigt = apool.tile([128, CB * 3], _f32)  # partition-0 recip scratch

            # warm the Exp table and the PE activity monitor while the first
            # supertiles stream in; the warm-up weight is a memset tile so the
            # burst starts without waiting on any DMA
            warm = cpool.tile([128, 8], _f32)
            nc.vector.memset(warm[:], 0.0)
            nc.scalar.activation(warm[:], warm[:], act.Exp)
            wz = cpool.tile([128, 128], mybir.dt.bfloat16)
            nc.vector.memset(wz[:], 0.0)
            for _ in range(NWARM):
                wps = ppool.tile([128, TILE_F], _f32, tag="ps")
                nc.tensor.matmul(
                    out=wps[:, :128], lhsT=wz[:], rhs=wz[:],
                    start=True, stop=True,
                )

            # each supertile ships as two column-half DMAs, one per HWDGE
            # ring, so both rings advance the same column position in lockstep
            supers = []
            for i, (c0, c1) in enumerate(sts):
                xt = xpool.tile([128, SUP], _f8, tag="xin")
                w = c1 - c0
                wh = _ceil_div(w // 2, 16) * 16
                nc.sync.dma_start(out=xt[:, :wh], in_=xd[:, c0 : c0 + wh])
                nc.scalar.dma_start(out=xt[:, wh:w], in_=xd[:, c0 + wh : c1])
                supers.append(xt)

            def finish_block(b, p0, P):
                fs = slice(b * TILE_F, (b + 1) * TILE_F)
                v = osb[p0:P, fs].rearrange("p (q c) -> p q c", c=CHD)
                o = obf[p0:P, fs].rearrange("p (q c) -> p q c", c=CHD)
                cv = csb[p0:P, b * (CB * NK) : (b + 1) * (CB * NK)].rearrange(
                    "p (q k) -> p q k", k=NK
                )
                P = P - p0
                # xy = sum*(rec*s) + grid*s  (gpsimd: SBUF-only ops, keeps the
                # vector queue free for the PSUM copies and the sigmoid chain)
                nc.gpsimd.tensor_tensor(
                    out=o[:, :, 0:2], in0=v[:, :, 0:2],
                    in1=cv[:, :, 2:3].to_broadcast((P, CB, 2)), op=alu.mult,
                )
                nc.gpsimd.tensor_tensor(
                    out=o[:, :, 0:2], in0=o[:, :, 0:2],
                    in1=cv[:, :, 0:2], op=alu.add,
                )
                # wh = exp(sum*rec) * s   (means are < 10, clip never binds)
                nc.vector.tensor_tensor(
                    out=v[:, :, 2:4], in0=v[:, :, 2:4],
                    in1=cv[:, :, 3:4].to_broadcast((P, CB, 2)), op=alu.mult,
                )
                nc.scalar.activation(v[:, :, 2:4], v[:, :, 2:4], act.Exp)
                nc.gpsimd.tensor_tensor(
                    out=o[:, :, 2:4], in0=v[:, :, 2:4],
                    in1=cv[:, :, 4:5].to_broadcast((P, CB, 2)), op=alu.mult,
                )
                # sigmoid(m) = 1 / (1 + exp(-m)); obj/cls pre-negated
                nc.vector.tensor_tensor(
                    out=v[:, :, 4:7], in0=v[:, :, 4:7],
                    in1=cv[:, :, 3:4].to_broadcast((P, CB, 3)), op=alu.mult,
                )
                nc.scalar.activation(v[:, :, 4:7], v[:, :, 4:7], act.Exp)
                # the custom-DVE reciprocal requires partition-base-0 APs, so
                # the +1 lands in a base-0 scratch and the copy shifts back
                sg = sigt[0:P, :].rearrange("p (q c) -> p q c", c=3)
                nc.vector.tensor_tensor(
                    out=sg, in0=v[:, :, 4:7],
                    in1=cv[:, :, 5:6].to_broadcast((P, CB, 3)), op=alu.add,
                )
                nc.vector.reciprocal_approx_fast(out=sg, in_=sg)
                nc.vector.tensor_copy(out=o[:, :, 4:7], in_=sg)
                nc.sync.dma_start(
                    out=outd[p0 : p0 + P, fs], in_=obf[p0 : p0 + P, fs]
                )

            for g in range(ng):
                ps = ppool.tile([128, TILE_F], _f32, tag="ps")
                for si, soff, ooff, wc, fstart, fstop in prog[g]:
                    nc.tensor.matmul(
                        out=ps[:, ooff : ooff + wc],
                        lhsT=wsb[:],
                        rhs=supers[si][:, soff : soff + wc],
                        start=fstart,
                        stop=fstop,
                    )
                pb = (g % GPB) * G
                b = g // GPB
                nc.vector.tensor_copy(
                    out=osb[pb : pb + G, b * TILE_F : (b + 1) * TILE_F],
                    in_=ps[0:G, :],
                )
                if g == ng - 1 or g % GPB == GPB - 1:
                    finish_block(b, 0, min(GPB, ng - b * GPB) * G)
    nc.compile()
    return nc


def _assemble(meta, outs):
    """Host-side gather of the per-core device outputs into [B, A, 7]."""
    total_a = sum(h * w for h, w in GRIDS)
    oc = np.stack(outs).astype(np.float32)  # [NCORES, 128, nb*TILE_F]
    ch = np.arange(COUT, dtype=np.int64)
    vals = oc[
        meta["coc"][:, None], meta["prow"][:, None], meta["fcol"][:, None] + ch
    ]
    final = np.empty((B, total_a, COUT), np.float32)
    final[meta["bcell"], meta["anch"]] = vals
    return final


def _run(inputs, trace=False, trace_cores=None):
    meta, in_maps = _prep(inputs)
    nc = _build(meta)
    kwargs = {}
    if trace:
        kwargs = dict(trace=True)
        if trace_cores is not None:
            kwargs["trace_cores"] = trace_cores
    res = run_bass_kernel_spmd(
        nc, in_maps, core_ids=list(range(NCORES)), **kwargs
    )
    out = _assemble(meta, [r["out"] for r in res.results])
    return out, res


def kernel(**inputs) -> np.ndarray:
    out, _ = _run(inputs, trace=False)
    return out
